# revision 34
# baseline (speedup 1.0000x reference)
"""Trainium2 Bass kernel: multi-head attention (B=4, T=2048, D=2048, H=16).

Sharding: 8 cores = 4 batches x 2 head-groups (tensor-parallel heads, data-
parallel batch). Each core handles one batch and 8 heads (f-slice of 1024
columns of the QKV projections / rows of the out-projection). Host sums the
two partial out-projection results per batch and adds the output bias.

v5: q/k projections in fp8(e4m3) DoubleRow (2x PE throughput; quantization
host-side, clip to +-240 = TRN e4m3 max, dequant folded into the PSUM
evacuation). x8 blocks loaded once and shared by q and k passes. The V
projection streams Wv in two F-halves (heads 0-3 then 4-7) so the P^T and
S^T pools can already be live: S^T+exp for heads 0-1 are woven between the
V chains, hiding ~36us of Scalar-engine exp under the V pass. After V, a
PV catch-up burst for head 0 restores the software pipeline: S^T matmuls
of sequence step i interleave with PV/normalize/transpose of step i-1 so
exp (the attention bottleneck) never starves. The out-projection for the
first t-half is woven into the second attention half; the rest runs last
with deeper PSUM buffering.
"""

import sys

if "/opt/trn_rl_repo" not in sys.path:
    sys.path.insert(0, "/opt/trn_rl_repo")

import numpy as np
import ml_dtypes

D = 2048          # d_model
T = 2048          # sequence length
B = 4             # batch
H = 16            # total heads
DH = 128          # head dim
GROUPS = 2        # head groups (tensor-parallel factor per batch)
HG = H // GROUPS  # heads per core = 8
F = HG * DH       # per-core projection width = 1024
P = 128
DC = D // P       # 16 contraction chunks
TC = T // P       # 16 t chunks
NCORES = 8
SCALE = float(1.0 / np.sqrt(DH))

SX = 32.0         # x fp8 scale (|x|max ~5.4 -> 173 < 240)
SW = 8192.0       # Wq/Wk/Wv fp8 scale (|W|max ~0.0221 -> 181 < 240)
DEQ = float(1.0 / (SX * SW))
NF8 = 4           # V-projection contraction chunks computed in fp8 DR
VSPLIT = (DC - NF8) * P

_PROGRAM = None


def _build_program():
    import concourse.bass as bass
    import concourse.tile as tile
    from concourse import bacc, mybir
    from concourse.bass import ts, ds
    from concourse.masks import make_identity

    bf16 = mybir.dt.bfloat16
    fp8 = mybir.dt.float8e4
    f32 = mybir.dt.float32
    DR = mybir.MatmulPerfMode.DoubleRow

    nc = bacc.Bacc("TRN2", target_bir_lowering=False, debug=False,
                   num_devices=NCORES)

    xT_d = nc.dram_tensor("xT", [DC, P, T], bf16, kind="ExternalInput")
    x8_d = nc.dram_tensor("x8", [DC, P, T], fp8, kind="ExternalInput")
    wq_d = nc.dram_tensor("wq", [DC, P, F], fp8, kind="ExternalInput")
    wk_d = nc.dram_tensor("wk", [DC, P, F], fp8, kind="ExternalInput")
    wv_d = nc.dram_tensor("wv", [DC, P, F], bf16, kind="ExternalInput")
    wv8_d = nc.dram_tensor("wv8", [NF8, P, F], fp8, kind="ExternalInput")
    wo_d = nc.dram_tensor("wo", [HG, P, D], bf16, kind="ExternalInput")
    bq_d = nc.dram_tensor("bq", [P, HG], f32, kind="ExternalInput")
    bk_d = nc.dram_tensor("bk", [P, HG], f32, kind="ExternalInput")
    bv_d = nc.dram_tensor("bv", [P, HG], f32, kind="ExternalInput")
    out_d = nc.dram_tensor("out", [DC, P, T], bf16, kind="ExternalOutput")

    Exp = mybir.ActivationFunctionType.Exp
    Identity = mybir.ActivationFunctionType.Identity

    with tile.TileContext(nc) as tc:
        from contextlib import ExitStack
        with ExitStack() as ctx:
            # ---- persistent pools (allocated first, live whole kernel) ----
            const = ctx.enter_context(tc.tile_pool(name="const", bufs=1))
            qkt = ctx.enter_context(tc.tile_pool(name="qkt", bufs=1))
            vpool = ctx.enter_context(tc.tile_pool(name="vpool", bufs=1))

            ident = const.tile([P, P], bf16, tag="ident")
            make_identity(nc, ident)
            zero_b = const.tile([P, 1], f32, tag="zerob")
            nc.vector.memset(zero_b[:], 0.0)
            bq_sb = const.tile([P, HG], f32, tag="bq")
            bk_sb = const.tile([P, HG], f32, tag="bk")
            bv_sb = const.tile([P, HG], f32, tag="bv")
            nc.sync.dma_start(bq_sb[:], bq_d[:])
            nc.sync.dma_start(bk_sb[:], bk_d[:])
            nc.sync.dma_start(bv_sb[:], bv_d[:])

            qT = [qkt.tile([P, T], bf16, tag=f"qT{h}", name=f"qT{h}")
                  for h in range(HG)]
            kT = [qkt.tile([P, T], bf16, tag=f"kT{h}", name=f"kT{h}")
                  for h in range(HG)]
            v_sb = vpool.tile([P, TC, HG, DH + 1], bf16, tag="v")

            nc.vector.memset(qT[0][:, 0:1], 0.0)
            nc.vector.memset(v_sb[:, :, :, DH:DH + 1], 1.0)

            # ---------------- Phase A: q/k projections (fp8 DR) -----------
            with tc.tile_pool(name="w8p", bufs=1) as w8p, \
                 tc.tile_pool(name="x8p", bufs=4) as x8p, \
                 tc.tile_pool(name="ps_qk", bufs=4, space="PSUM") as ps_qk:
                wq_sb = w8p.tile([P, DC, F], fp8, tag="wq8")
                wk_sb = w8p.tile([P, DC, F], fp8, tag="wk8")
                x8t = [x8p.tile([P, DC, 512], fp8, tag="x8blk",
                                name=f"x8blk{tcb}") for tcb in range(4)]
                # the opening chain is DMA-issue-rate bound: interleave wq
                # (2-chunk) with x8 block 0 (4-chunk) as few large transfers
                # so everything the first t-block needs lands in ~15us, then
                # x8 block 1 before the wk bulk
                src0 = x8_d[:, :, ds(0, 512)].rearrange("c p t -> p c t")
                wqr = wq_d[:].rearrange("c p f -> p c f")
                wkr = wk_d[:].rearrange("c p f -> p c f")
                for j in range(4):
                    nc.sync.dma_start(wq_sb[:, ds(4 * j, 2)],
                                      wqr[:, ds(4 * j, 2)])
                    nc.sync.dma_start(wq_sb[:, ds(4 * j + 2, 2)],
                                      wqr[:, ds(4 * j + 2, 2)])
                    nc.sync.dma_start(x8t[0][:, ds(4 * j, 4)],
                                      src0[:, ds(4 * j, 4)])
                src1 = x8_d[:, :, ds(512, 512)].rearrange("c p t -> p c t")
                for dg in range(0, 16, 4):
                    nc.sync.dma_start(x8t[1][:, ds(dg, 4)],
                                      src1[:, ds(dg, 4)])
                for tcb in range(2, 4):
                    src = x8_d[:, :, ds(tcb * 512, 512)].rearrange(
                        "c p t -> p c t")
                    for dg in range(0, 16, 4):
                        nc.sync.dma_start(x8t[tcb][:, ds(dg, 4)],
                                          src[:, ds(dg, 4)])
                for dg in range(0, 16, 2):
                    nc.sync.dma_start(wk_sb[:, ds(dg, 2)],
                                      wkr[:, ds(dg, 2)])

                for w_sb, bias_sb, dst in ((wq_sb, bq_sb, qT),
                                           (wk_sb, bk_sb, kT)):
                    for tcb in range(4):
                        for h in range(HG):
                            ps = ps_qk.tile([P, 512], f32, tag="ps512",
                                            name=f"ps{tcb}{h}")
                            for dc in range(0, DC, 2):
                                nc.tensor.matmul(
                                    ps[:],
                                    w_sb[:, dc:dc + 2, ds(h * DH, DH)],
                                    x8t[tcb][:, dc:dc + 2],
                                    start=(dc == 0), stop=(dc == DC - 2),
                                    perf_mode=DR)
                            nc.scalar.activation(
                                dst[h][:, ds(tcb * 512, 512)], ps[:],
                                Identity, bias=bias_sb[:, ds(h, 1)],
                                scale=DEQ)

            # ---- attention S^T/P^T pools (live from the V pass to the end)
            ptpool = ctx.enter_context(tc.tile_pool(name="ptpool", bufs=2))
            ps_st = ctx.enter_context(
                tc.tile_pool(name="ps_st", bufs=2, space="PSUM"))

            pt_tiles = {}

            def s_unit(i, kc):
                """S^T matmuls + exp for one k-chunk of sequence step i."""
                half, h = divmod(i, HG)
                q0 = half * (T // 2)
                pt = pt_tiles[i]
                st = ps_st.tile([P, T // 2], f32, tag="st",
                                name=f"st{i}_{kc}")
                for qc in range(2):
                    nc.tensor.matmul(
                        st[:, ds(qc * 512, 512)],
                        kT[h][:, ds(kc * P, P)],
                        qT[h][:, ds(q0 + qc * 512, 512)],
                        start=True, stop=True)
                nc.scalar.activation(pt[:, kc], st[:], Exp,
                                     bias=zero_b[:, :], scale=SCALE)

            # -------- V pass (paired psl/psr chains) + S/exp of heads 0-1
            with tc.tile_pool(name="wvp", bufs=1) as wvp, \
                 tc.tile_pool(name="xv", bufs=2) as xvp, \
                 tc.tile_pool(name="ps_v", bufs=3, space="PSUM") as ps_v:
                wv_sb = wvp.tile([P, DC, F], bf16, tag="wv")
                wv8_sb = wvp.tile([P, NF8, F], fp8, tag="wv8")
                for dc in range(DC - NF8):
                    nc.sync.dma_start(wv_sb[:, dc], wv_d[dc])
                for c4 in range(NF8):
                    nc.sync.dma_start(wv8_sb[:, c4], wv8_d[c4])
                pt_tiles[0] = ptpool.tile([P, TC, T // 2], bf16, tag="pt",
                                          name="pt0")
                pt_tiles[1] = ptpool.tile([P, TC, T // 2], bf16, tag="pt",
                                          name="pt1")
                sunits = [(i, kc) for i in range(2) for kc in range(TC)]
                sidx = 0
                # front-load a few S/exp units to cover the wv DMA
                while sidx < 4:
                    s_unit(*sunits[sidx])
                    sidx += 1
                for blk in range(TC):  # t-blocks of 128, double-buffered
                    xblk = xvp.tile([P, DC, P], bf16, tag="xvblk",
                                    name=f"xvblk{blk}")
                    src = xT_d[:, :, ds(blk * P, P)].rearrange(
                        "c p t -> p c t")
                    for dg in range(0, 16, 4):
                        nc.sync.dma_start(xblk[:, ds(dg, 4)],
                                          src[:, ds(dg, 4)])
                    x8blk = xvp.tile([P, NF8, P], fp8, tag="x8vblk",
                                     name=f"x8vblk{blk}")
                    src8 = x8_d[ds(DC - NF8, NF8), :,
                                ds(blk * P, P)].rearrange("c p t -> p c t")
                    nc.sync.dma_start(x8blk[:], src8[:])
                    tc_ = blk
                    psl = ps_v.tile([P, 512], f32, tag="psv",
                                    name=f"psl{tc_}")
                    psr = ps_v.tile([P, 512], f32, tag="psv",
                                    name=f"psr{tc_}")
                    for dc in range(DC - NF8):
                        lhs = xblk[:, dc]
                        nc.tensor.matmul(
                            psl[:], lhs, wv_sb[:, dc, 0:512],
                            start=(dc == 0), stop=False)
                        nc.tensor.matmul(
                            psr[:], lhs, wv_sb[:, dc, 512:1024],
                            start=(dc == 0), stop=False)
                    for pp in range(0, NF8, 2):
                        lhs8 = x8blk[:, pp:pp + 2]
                        nc.tensor.matmul(
                            psl[:], lhs8, wv8_sb[:, pp:pp + 2, 0:512],
                            start=False, stop=(pp == NF8 - 2),
                            perf_mode=DR)
                        nc.tensor.matmul(
                            psr[:], lhs8, wv8_sb[:, pp:pp + 2, 512:1024],
                            start=False, stop=(pp == NF8 - 2),
                            perf_mode=DR)
                    nc.scalar.activation(
                        v_sb[:, tc_, 0:4, 0:DH],
                        psl[:].rearrange("p (h d) -> p h d", d=DH),
                        Identity, scale=DEQ)
                    nc.scalar.activation(
                        v_sb[:, tc_, 4:8, 0:DH],
                        psr[:].rearrange("p (h d) -> p h d", d=DH),
                        Identity, scale=DEQ)
                    # weave S/exp units between V chain pairs
                    for _ in range(2):
                        if sidx < len(sunits):
                            s_unit(*sunits[sidx])
                            sidx += 1

            # ------- Phase B: attention (half-major, software-pipelined) ----
            with tc.tile_pool(name="ytp", bufs=1) as ytp, \
                 tc.tile_pool(name="wop", bufs=3) as wop, \
                 tc.tile_pool(name="osb", bufs=4) as osb:
                yT = ytp.tile([P, HG, T], bf16, tag="yT")
                nc.vector.memset(yT[:, 0, 0:1], 0.0)

                wo_tiles = {}

                def load_wo(dch):
                    wo_t = wop.tile([P, HG, P], bf16, tag="wo",
                                    name=f"wo{dch}")
                    nc.sync.dma_start(
                        wo_t[:],
                        wo_d[:, :, ds(dch * P, P)].rearrange("h p d -> p h d"))
                    wo_tiles[dch] = wo_t

                def c_chain(dch, tcb, pool):
                    wo_t = wo_tiles[dch]
                    pso = pool.tile([P, 512], f32, tag="pso",
                                    name=f"pso{dch}_{tcb}")
                    for fc in range(HG):
                        nc.tensor.matmul(
                            pso[:], wo_t[:, fc],
                            yT[:, fc, ds(tcb * 512, 512)],
                            start=(fc == 0), stop=(fc == HG - 1))
                    ot = osb.tile([P, 512], bf16, tag="ot",
                                  name=f"ot{dch}_{tcb}")
                    nc.vector.tensor_copy(ot[:], pso[:])
                    nc.sync.dma_start(out_d[dch, :, ds(tcb * 512, 512)],
                                      ot[:])

                bscope = ExitStack()
                ystage = bscope.enter_context(
                    tc.tile_pool(name="ystage", bufs=2))
                rspool = bscope.enter_context(
                    tc.tile_pool(name="rspool", bufs=2))
                ps_pv = bscope.enter_context(
                    tc.tile_pool(name="ps_pv", bufs=2, space="PSUM"))
                ps_tr = bscope.enter_context(
                    tc.tile_pool(name="ps_tr", bufs=1, space="PSUM"))

                def pv_unit(i, qs):
                    """PV chain + normalize + transpose + bias for seq i."""
                    half, h = divmod(i, HG)
                    q0 = half * (T // 2)
                    pt = pt_tiles[i]
                    pv = ps_pv.tile([P, DH + 1], f32, tag="pv",
                                    name=f"pv{i}_{qs}")
                    for kc in range(TC):
                        nc.tensor.matmul(
                            pv[:], pt[:, kc, ds(qs * P, P)], v_sb[:, kc, h],
                            start=(kc == 0), stop=(kc == TC - 1))
                    rs = rspool.tile([P, 1], f32, tag="rs",
                                     name=f"rs{i}_{qs}")
                    nc.vector.reciprocal(rs[:], pv[:, DH:DH + 1])
                    yst = ystage.tile([P, P], bf16, tag="yst",
                                      name=f"yst{i}_{qs}")
                    nc.vector.tensor_scalar_mul(yst[:], pv[:, 0:DH], rs[:])
                    tr = ps_tr.tile([P, P], bf16, tag="tr",
                                    name=f"tr{i}_{qs}")
                    nc.tensor.transpose(tr[:], yst[:], ident[:])
                    nc.vector.tensor_scalar_add(
                        yT[:, h, ds(q0 + qs * P, P)], tr[:],
                        bv_sb[:, ds(h, 1)])

                # catch-up: PV for head 0 (its S/exp ran inside the V pass)
                for qs in range(8):
                    pv_unit(0, qs)

                # steady state: S/exp of step i + PV units of step i-1
                for i in range(2, 2 * HG):
                    pt_tiles[i] = ptpool.tile([P, TC, T // 2], bf16,
                                              tag="pt", name=f"pt{i}")
                    if i == 8:
                        load_wo(0)
                        load_wo(1)
                    for kc in range(TC):
                        s_unit(i, kc)
                        if kc % 2 == 1:
                            pv_unit(i - 1, (kc - 1) // 2)
                        if i >= 9 and kc % 4 == 3:
                            widx = (i - 9) * 4 + kc // 4  # 0..27
                            dch, tcb = divmod(widx, 2)
                            c_chain(dch, tcb, ps_tr)
                            if tcb == 1 and dch < DC - 3:
                                load_wo(dch + 2)
                # trailing pv units for the last step
                for qs in range(8):
                    pv_unit(2 * HG - 1, qs)

                bscope.close()

                # ---- Phase C tail: remaining out-proj ----
                with tc.tile_pool(name="ps_o2", bufs=4,
                                  space="PSUM") as ps_o2:
                    # leftover t-half-0 chains (dch 14,15)
                    load_wo(14)
                    load_wo(15)
                    for dch in (14, 15):
                        for tcb in (0, 1):
                            c_chain(dch, tcb, ps_o2)
                    load_wo(0)
                    for dch in range(DC):
                        if dch < DC - 1:
                            load_wo(dch + 1)
                        for tcb in (2, 3):
                            c_chain(dch, tcb, ps_o2)

    nc.compile()
    return nc


def _get_program():
    global _PROGRAM
    if _PROGRAM is None:
        _PROGRAM = _build_program()
    return _PROGRAM


def _q8(a, scale):
    """TRN-safe e4m3 quantization (clip to +-240)."""
    return np.clip(np.asarray(a, np.float32) * scale, -240.0, 240.0).astype(
        ml_dtypes.float8_e4m3fn)


def _prep_inputs(x, Wq, bq, Wk, bk, Wv, bv, Wo, bo):
    """Build the 8 per-core input maps (host-side sharding, free)."""
    bf = ml_dtypes.bfloat16
    x = np.asarray(x, dtype=np.float32)
    WqT = np.ascontiguousarray(np.asarray(Wq, np.float32).T)  # [D, D]
    WkT = np.ascontiguousarray(np.asarray(Wk, np.float32).T)
    WvT = np.ascontiguousarray(np.asarray(Wv, np.float32).T)
    WoT = np.ascontiguousarray(np.asarray(Wo, np.float32).T)  # [D, D] (f, d)

    in_maps = []
    for c in range(NCORES):
        b, g = divmod(c, GROUPS)
        fsl = slice(g * F, (g + 1) * F)
        xTb = np.ascontiguousarray(x[b].T)                    # [D, T] f32
        m = {
            # xT and wv are pre-scaled by the fp8 scales (exact powers of 2)
            # so bf16 and fp8-DR chunks of the V projection accumulate in the
            # same PSUM scale; the evacuation applies the 1/(SX*SW) dequant.
            "xT": (xTb * SX).astype(bf).reshape(DC, P, T),
            "x8": _q8(xTb, SX).reshape(DC, P, T),
            "wq": _q8(WqT[:, fsl], SW).reshape(DC, P, F),
            "wk": _q8(WkT[:, fsl], SW).reshape(DC, P, F),
            "wv": (np.ascontiguousarray(WvT[:, fsl]) * SW).astype(bf).reshape(
                DC, P, F),
            "wv8": _q8(WvT[VSPLIT:, fsl], SW).reshape(NF8, P, F),
            "wo": np.ascontiguousarray(WoT[fsl, :]).astype(bf).reshape(
                HG, P, D),
            "bq": np.ascontiguousarray(
                np.asarray(bq, np.float32)[fsl].reshape(HG, P).T),
            "bk": np.ascontiguousarray(
                np.asarray(bk, np.float32)[fsl].reshape(HG, P).T),
            "bv": np.ascontiguousarray(
                np.asarray(bv, np.float32)[fsl].reshape(HG, P).T),
        }
        in_maps.append(m)
    return in_maps


def _combine(results, bo):
    bo = np.asarray(bo, np.float32)
    out = np.empty((B, T, D), dtype=np.float32)
    for b in range(B):
        oT = (results[b * GROUPS]["out"].reshape(D, T).astype(np.float32)
              + results[b * GROUPS + 1]["out"].reshape(D, T).astype(np.float32))
        out[b] = oT.T + bo[None, :]
    return out


def kernel(x, Wq, bq, Wk, bk, Wv, bv, Wo, bo):
    from concourse.bass_utils import run_bass_kernel_spmd

    nc = _get_program()
    in_maps = _prep_inputs(x, Wq, bq, Wk, bk, Wv, bv, Wo, bo)
    res = run_bass_kernel_spmd(nc, in_maps, list(range(NCORES))).results
    return _combine(res, bo)
